# revision 1
# baseline (speedup 1.0000x reference)
"""PointPillarScatter (intersweep, 3 bins) Trainium2 Bass kernel.

Problem: for each of 3 bins, scatter 64000 pillar rows [64 feats] into a
[B=4, C=64, NY=496, NX=432] canvas at (b, :, y, x); empty cells are zero.

Strategy (8 NeuronCores, SPMD):
  - The output is 658 MB, ~92.5% zeros -> write-bandwidth bound. Dense
    output tiles (zeros included) are generated on-chip and stored with
    large contiguous DMAs.
  - Host-side (cheap numpy): shard the 12 (bin, b) canvases into 48
    quarter-canvases of 124 y-rows; 6 per core, processed as 3 pairs.
    Each quarter is cut into 248 windows of 216 cells; a window's pillars
    (max 19) go into RP=21 padded slots.
  - Per window one bf16 matmul places the pillar features:
      out[128, 216] = lhsT[128, 128].T @ onehot[128, 216]
    K rows hold [3 split terms x 2 quarters x 21 slots] (+2 zero rows);
    the fp32 features are split into three bf16 terms whose fp32 sum is
    exactly the original value (verified), so the K-dim reduction
    reconstructs fp32 bit-exactly with NO PSUM accumulation flags --
    accumulating matmuls (start=False) run at half PE rate, and K<128
    matmuls also run at half rate, so both are avoided.
    onehot[k, c] = (x_rel[k] == c), one DVE/GPSIMD tensor_scalar
    is_equal against an iota row; rows are replicated across the 3 terms.
  - lhsT is block-"diagonal" over quarters (A in M-cols 0:64, B in
    64:128). Only those blocks are DMA'd (compact loads into persistent
    pre-zeroed SBUF tiles); zero regions are never rewritten.
  - Two windows share one PSUM bank; one copy (ACT/DVE) moves
    [128, 432] to SBUF staging; one [128 x 53568 B] contiguous DMA per
    62-window chunk writes out. Host de-interleaves the quarters.
  - in-DMAs ride the scalar-engine HWDGE ring so they never queue behind
    the ~20 us out-DMAs on the sync-engine ring (FIFO per ring).
"""

import numpy as np
import ml_dtypes

import concourse.bass as bass
import concourse.tile as tile
from concourse import bacc, mybir
from concourse.bass_utils import run_bass_kernel_spmd

# Problem geometry (hardcoded; kernel.py must be self-contained).
B = 4
C = 64
NX = 432
NY = 496
NBINS = 3
NCORES = 8

NQ = NBINS * B * 4  # 48 quarter-canvases
YQ = NY // 4  # 124 y-rows per quarter
QCELLS = YQ * NX  # 53568 cells per quarter
QPC = NQ // NCORES  # 6 quarters per core
PAIRS = QPC // 2  # 3 pairs per core
NW = 216  # cells per window
WPQ = QCELLS // NW  # 248 windows per quarter
WINDOWS = PAIRS * WPQ  # 744 windows per core
CH = 62  # windows per staging chunk
NCHUNKS = WPQ // CH  # 4 chunks per pair
NTERMS = 3  # bf16 split terms (exact fp32 reconstruction)
RP = 21  # padded pillar slots per window per quarter (max count is 19)
KROWS = 128  # NTERMS*2*RP = 126, padded to 128 (K<128 MMs run half-rate)

MASK_GPSIMD_EVERY = 3  # every 3rd mask on GPSIMD, rest DVE
COPY_DVE_EVERY = 3  # every 3rd PSUM->SBUF copy on DVE, rest ACT

_cache = {}


def _build():
    nc = bacc.Bacc(trn_type="TRN2")
    bf16 = mybir.dt.bfloat16
    f32 = mybir.dt.float32
    lhst_d = nc.dram_tensor("lhst", [KROWS, WINDOWS, C], bf16,
                            kind="ExternalInput")
    iota_d = nc.dram_tensor("iotat", [KROWS, NW], bf16, kind="ExternalInput")
    relc_d = nc.dram_tensor("relc", [KROWS, WINDOWS], f32,
                            kind="ExternalInput")
    out_d = nc.dram_tensor("out", [PAIRS, NCHUNKS, 128, CH, NW], f32,
                           kind="ExternalOutput")

    with tile.TileContext(nc) as tc:
        with (
            tc.tile_pool(name="const", bufs=1) as constp,
            tc.tile_pool(name="cpt", bufs=2) as cptp,
            tc.tile_pool(name="lhstp", bufs=1) as lhstp,
            tc.tile_pool(name="maskp", bufs=6) as maskp,
            tc.tile_pool(name="stage", bufs=2) as stagep,
            tc.tile_pool(name="psum", bufs=3, space=bass.MemorySpace.PSUM) as psump,
        ):
            iota = constp.tile([KROWS, NW], bf16, name="iota")
            relc = constp.tile([KROWS, WINDOWS], f32, name="relc")
            nc.scalar.dma_start(out=iota[:], in_=iota_d[:])
            nc.scalar.dma_start(out=relc[:], in_=relc_d[:])
            # persistent ping-pong stationary tiles; zero regions (pad rows,
            # off-diagonal col blocks) are memset once and never rewritten
            lts = [lhstp.tile([KROWS, CH, 128], bf16, name=f"lt{pp}",
                              tag=f"lt{pp}") for pp in range(2)]
            for pp in range(2):
                nc.gpsimd.memset(lts[pp][:], 0.0)
            gw = 0
            for pair in range(PAIRS):
                for ch in range(NCHUNKS):
                    g0 = pair * WPQ + ch * CH
                    pp = (pair * NCHUNKS + ch) % 2
                    lt = lts[pp]
                    cpt = cptp.tile([KROWS, CH, C], bf16, name="cpt")
                    nc.scalar.dma_start(out=cpt[:], in_=lhst_d[:, g0:g0 + CH, :])
                    nc.gpsimd.tensor_copy(out=lt[0:64, :, 0:C], in_=cpt[0:64])
                    nc.gpsimd.tensor_copy(out=lt[64:128, :, C:128],
                                          in_=cpt[64:128])
                    st = stagep.tile([128, CH, NW], f32, name="st")
                    for j in range(CH // 2):
                        # [128, 2, 512] spans TWO PSUM banks: the paired
                        # matmuls hit different banks so they pipeline
                        acc = psump.tile([128, 2, 512], f32, name="acc")
                        for par in range(2):
                            w = 2 * j + par
                            mask = maskp.tile([KROWS, NW], bf16, name="mask")
                            nc.vector.tensor_scalar(
                                out=mask[:],
                                in0=iota[:],
                                scalar1=relc[:, g0 + w:g0 + w + 1],
                                scalar2=None,
                                op0=mybir.AluOpType.is_equal,
                            )
                            nc.tensor.matmul(acc[:, par, 0:NW], lt[:, w, :],
                                             mask[:], start=True, stop=True)
                            gw += 1
                        nc.scalar.copy(st[:, 2 * j:2 * j + 2, :],
                                       acc[:, :, 0:NW])
                    nc.sync.dma_start(out=out_d[pair, ch], in_=st[:])
    nc.compile()
    return nc


def _split3(feats):
    """Split fp32 features into 3 bf16 terms summing exactly to the input."""
    h1 = feats.astype(ml_dtypes.bfloat16)
    r1 = feats - h1.astype(np.float32)
    h2 = r1.astype(ml_dtypes.bfloat16)
    h3 = (r1 - h2.astype(np.float32)).astype(ml_dtypes.bfloat16)
    rec = (h1.astype(np.float32) + h2.astype(np.float32)) + h3.astype(np.float32)
    if not (rec == feats).all():
        raise FloatingPointError("bf16 3-term split not exact")
    return h1, h2, h3


def _pack(inputs):
    lhst = np.zeros((NCORES, KROWS, WINDOWS, C), ml_dtypes.bfloat16)
    iota = np.broadcast_to(np.arange(NW, dtype=np.float32),
                           (NCORES, KROWS, NW)).astype(ml_dtypes.bfloat16)
    relc_a = np.full((NCORES, KROWS, WINDOWS), -1.0, np.float32)

    for bin_i in range(NBINS):
        feats = np.asarray(inputs[f"pillar_features_bin_{bin_i}"], np.float32)
        terms = _split3(feats)
        coords = np.asarray(inputs[f"voxel_coords_bin_{bin_i}"])
        cb = np.asarray(coords[:, 0], np.int64)
        cy = np.asarray(coords[:, 2], np.int64)
        cx = np.asarray(coords[:, 3], np.int64)
        for b in range(B):
            rows_b = np.nonzero(cb == b)[0]
            y_b, x_b = cy[rows_b], cx[rows_b]
            for yq in range(4):
                q = bin_i * 16 + b * 4 + yq
                core, j = divmod(q, QPC)
                pair, half = divmod(j, 2)
                sel = (y_b >= YQ * yq) & (y_b < YQ * (yq + 1))
                rows = rows_b[sel]
                qcell = (y_b[sel] - YQ * yq) * NX + x_b[sel]
                w = qcell // NW
                rel = qcell % NW
                order = np.argsort(w, kind="stable")
                rows, w, rel = rows[order], w[order], rel[order]
                cnt = np.bincount(w, minlength=WPQ)
                if cnt.max() > RP:
                    raise OverflowError(int(cnt.max()))
                off = np.concatenate([[0], np.cumsum(cnt)[:-1]])
                slot = np.arange(len(rows)) - off[w]
                wins = pair * WPQ + w
                for t in range(NTERMS):
                    r = half * 64 + t * RP + slot
                    lhst[core, r, wins, :] = terms[t][rows]
                    relc_a[core, r, wins] = rel
    return [{"lhst": lhst[c], "iotat": iota[c], "relc": relc_a[c]}
            for c in range(NCORES)]


def _run(inputs, trace=False):
    if "nc" not in _cache:
        _cache["nc"] = _build()
    nc = _cache["nc"]
    in_maps = _pack(inputs)
    res = run_bass_kernel_spmd(nc, in_maps, core_ids=list(range(NCORES)),
                               trace=trace)
    outs = [np.zeros((B, C, NY, NX), np.float32) for _ in range(NBINS)]
    for q in range(NQ):
        bin_i, rem = divmod(q, 16)
        b, yq = divmod(rem, 4)
        core, j = divmod(q, QPC)
        pair, half = divmod(j, 2)
        # [NCHUNKS, 64, CH, NW] -> [64, QCELLS] -> [64, YQ, NX]
        blk = res.results[core]["out"][pair, :, half * C:(half + 1) * C]
        outs[bin_i][b, :, YQ * yq:YQ * (yq + 1), :] = (
            blk.transpose(1, 0, 2, 3).reshape(C, YQ, NX))
    return tuple(outs), res


def kernel(**inputs):
    out, _ = _run(inputs)
    return out


def kernel_traced(**inputs):
    """Like kernel() but also returns BassKernelResults (for test.py)."""
    return _run(inputs, trace=True)



# revision 4
# speedup vs baseline: 2.8947x; 2.8947x over previous
"""PointPillarScatter (intersweep, 3 bins) Trainium2 Bass kernel.

Problem: for each of 3 bins, scatter 64000 pillar rows [64 feats] into a
[B=4, C=64, NY=496, NX=432] fp32 canvas at (b, :, y, x); empty cells zero.

Strategy (8 NeuronCores, SPMD), v2 — fp16 transport, no GPSIMD steady-state:
  - Output is 658 MB; tolerance is rel_err < 2e-2, so the canvas travels as
    fp16 (host upcasts): 41.1 MB out per core instead of 82.3 MB.
  - Features travel as a single fp16 term (quantization ~2.4e-4 rel): the
    3-term bf16 exact split of v1 tripled input bytes for precision the
    gate does not need.
  - Each core owns 6 quarter-canvases (124 y-rows x 432) = 3 pairs x 2
    halves. A pair is cut into 108 windows of 496 cells. Per window one
    fp16 matmul places both halves' pillars:
      psum[128, 496] = lhsT[128, 128].T @ onehot[128, 496]
    K rows = 2 halves x 64 slots (max occupancy measured 41). The one-hot
    is one DVE tensor_scalar is_equal (fp16 iota row vs per-row f32 x-pos;
    runs in 4x perf mode, ~350 ns).
  - v1's GPSIMD spread-copies held the DVE/GpSimd shared SBUF port for
    ~13 us each, blocking every DVE mask (2-port perf mode) -> 701 ns
    masks and a 536 us kernel. v2 keeps GPSIMD idle: lhsT diagonal blocks
    are DMA'd (HWDGE, strided SBUF writes) into persistent tiles whose
    off-diagonal zeros are memset once by the DVE at start.
  - PSUM tiles [128, 4, 512] f32 (4 banks; each window's 496 cols sit
    inside one bank). Evacuation PSUM->SBUF casts to fp16 4 windows at a
    time (FD=1984) and is split DVE/ACT to balance engine load (measured:
    DVE ~160ns + 1.05ns/elem, ACT ~625ns + 1.09ns/elem; DVE also carries
    the masks).
  - 12 out-DMAs of 3.43 MB (sync ring) overlap in-DMAs (scalar ring).
"""

import numpy as np
import ml_dtypes

import concourse.bass as bass
import concourse.tile as tile
from concourse import bacc, mybir
from concourse.bass_utils import run_bass_kernel_spmd

# Problem geometry (hardcoded; kernel.py must be self-contained).
B = 4
C = 64
NX = 432
NY = 496
NBINS = 3
NCORES = 8

NQ = NBINS * B * 4          # 48 quarter-canvases
YQ = NY // 4                # 124 y-rows per quarter
QCELLS = YQ * NX            # 53568 cells per quarter
QPC = NQ // NCORES          # 6 quarters per core
PAIRS = QPC // 2            # 3 pairs per core
NW = 496                    # cells per window (<=512: one PSUM bank)
WPP = QCELLS // NW          # 108 windows per pair
WINDOWS = PAIRS * WPP       # 324 windows per core
CH = 27                     # windows per staging chunk / out-DMA
NCHUNKS = WPP // CH         # 4 chunks per pair
RP = 64                     # pillar slots per window per half (max seen 41)
PSW = 4                     # windows per PSUM tile (4 banks)
NPT = (CH + PSW - 1) // PSW  # psum tiles per chunk (6x4 + 1x3)

# which psum-tile evacuations go on the DVE (rest on ACT): ~2 of 7
DVE_EVAC_T = (2, 5)

_cache = {}


def _build():
    nc = bacc.Bacc(trn_type="TRN2")
    fp16 = mybir.dt.float16
    f32 = mybir.dt.float32
    lhst_d = nc.dram_tensor("lhst", [2 * RP, WINDOWS, C], fp16,
                            kind="ExternalInput")
    iota_d = nc.dram_tensor("iotat", [2 * RP, NW], fp16, kind="ExternalInput")
    relc_d = nc.dram_tensor("relc", [2 * RP, WINDOWS], f32,
                            kind="ExternalInput")
    out_d = nc.dram_tensor("out", [PAIRS, NCHUNKS, 128, CH, NW], fp16,
                           kind="ExternalOutput")

    with tile.TileContext(nc) as tc:
        with (
            tc.tile_pool(name="const", bufs=1) as constp,
            tc.tile_pool(name="ltp", bufs=1) as ltp,
            tc.tile_pool(name="maskp", bufs=6) as maskp,
            tc.tile_pool(name="stage", bufs=2) as stagep,
            tc.tile_pool(name="psum", bufs=2, space=bass.MemorySpace.PSUM) as psump,
        ):
            iota = constp.tile([128, NW], fp16, name="iota")
            relc = constp.tile([128, WINDOWS], f32, name="relc")
            nc.scalar.dma_start(out=iota[:], in_=iota_d[:])
            nc.scalar.dma_start(out=relc[:], in_=relc_d[:])
            # persistent ping-pong stationary tiles; off-diagonal blocks are
            # zeroed once (DVE) and never rewritten -- chunk loads DMA only
            # the diagonal blocks.
            lts = [ltp.tile([128, CH, 128], fp16, name=f"lt{pp}",
                            tag=f"lt{pp}") for pp in range(2)]
            for pp in range(2):
                nc.vector.memset(lts[pp][:], 0.0)
            for pair in range(PAIRS):
                for ch in range(NCHUNKS):
                    g0 = pair * WPP + ch * CH
                    pp = (pair * NCHUNKS + ch) % 2
                    lt = lts[pp]
                    nc.scalar.dma_start(out=lt[0:RP, :, 0:C],
                                        in_=lhst_d[0:RP, g0:g0 + CH, :])
                    nc.scalar.dma_start(out=lt[RP:2 * RP, :, C:128],
                                        in_=lhst_d[RP:2 * RP, g0:g0 + CH, :])
                    st = stagep.tile([128, CH, NW], fp16, name="st")
                    for t in range(NPT):
                        nw = min(PSW, CH - PSW * t)
                        pt = psump.tile([128, PSW, 512], f32, name="pt")
                        for j in range(nw):
                            w = PSW * t + j
                            mask = maskp.tile([128, NW], fp16, name="mask")
                            nc.vector.tensor_scalar(
                                out=mask[:],
                                in0=iota[:],
                                scalar1=relc[:, g0 + w:g0 + w + 1],
                                scalar2=None,
                                op0=mybir.AluOpType.is_equal,
                            )
                            nc.tensor.matmul(pt[:, j, 0:NW], lt[:, w, :],
                                             mask[:], start=True, stop=True)
                        dst = st[:, PSW * t:PSW * t + nw, :]
                        src = pt[:, 0:nw, 0:NW]
                        if t in DVE_EVAC_T:
                            nc.vector.tensor_copy(out=dst, in_=src)
                        else:
                            nc.scalar.copy(out=dst, in_=src)
                    nc.sync.dma_start(out=out_d[pair, ch], in_=st[:])
    nc.compile()
    return nc


def _pack(inputs):
    lhst = np.zeros((NCORES, 2 * RP, WINDOWS, C), np.float16)
    iota = np.broadcast_to(np.arange(NW, dtype=np.float32),
                           (NCORES, 2 * RP, NW)).astype(np.float16)
    relc_a = np.full((NCORES, 2 * RP, WINDOWS), -1.0, np.float32)

    for bin_i in range(NBINS):
        feats = np.asarray(inputs[f"pillar_features_bin_{bin_i}"],
                           np.float32).astype(np.float16)
        coords = np.asarray(inputs[f"voxel_coords_bin_{bin_i}"])
        cb = np.asarray(coords[:, 0], np.int64)
        cy = np.asarray(coords[:, 2], np.int64)
        cx = np.asarray(coords[:, 3], np.int64)
        for b in range(B):
            rows_b = np.nonzero(cb == b)[0]
            y_b, x_b = cy[rows_b], cx[rows_b]
            for yq in range(4):
                q = bin_i * 16 + b * 4 + yq
                core, j = divmod(q, QPC)
                pair, half = divmod(j, 2)
                sel = (y_b >= YQ * yq) & (y_b < YQ * (yq + 1))
                rows = rows_b[sel]
                qcell = (y_b[sel] - YQ * yq) * NX + x_b[sel]
                w = qcell // NW
                rel = qcell % NW
                order = np.argsort(w, kind="stable")
                rows, w, rel = rows[order], w[order], rel[order]
                cnt = np.bincount(w, minlength=WPP)
                if cnt.max() > RP:
                    raise OverflowError(int(cnt.max()))
                off = np.concatenate([[0], np.cumsum(cnt)[:-1]])
                slot = np.arange(len(rows)) - off[w]
                wins = pair * WPP + w
                r = half * RP + slot
                lhst[core, r, wins, :] = feats[rows]
                relc_a[core, r, wins] = rel
    return [{"lhst": lhst[c], "iotat": iota[c], "relc": relc_a[c]}
            for c in range(NCORES)]


def _run(inputs, trace=False):
    if "nc" not in _cache:
        _cache["nc"] = _build()
    nc = _cache["nc"]
    in_maps = _pack(inputs)
    res = run_bass_kernel_spmd(nc, in_maps, core_ids=list(range(NCORES)),
                               trace=trace)
    outs = [np.zeros((B, C, NY, NX), np.float32) for _ in range(NBINS)]
    for core in range(NCORES):
        blk = np.asarray(res.results[core]["out"])  # [PAIRS,NCHUNKS,128,CH,NW]
        for pair in range(PAIRS):
            # [NCHUNKS, 128, CH, NW] -> [128, QCELLS]
            a = blk[pair].transpose(1, 0, 2, 3).reshape(128, QCELLS)
            for half in range(2):
                q = core * QPC + pair * 2 + half
                bin_i, rem = divmod(q, 16)
                b, yq = divmod(rem, 4)
                outs[bin_i][b, :, YQ * yq:YQ * (yq + 1), :] = (
                    a[half * C:(half + 1) * C]
                    .reshape(C, YQ, NX).astype(np.float32))
    return tuple(outs), res


def kernel(**inputs):
    out, _ = _run(inputs)
    return out


def kernel_traced(**inputs):
    """Like kernel() but also returns BassKernelResults (for test.py)."""
    return _run(inputs, trace=True)
